# revision 38
# baseline (speedup 1.0000x reference)
"""GAT 2-layer + FC Trainium2 kernel, 8-core SPMD — degree-sorted edition.

Sharding: nodes sorted by in-degree, grouped into 160 blocks of 128 dst
nodes; blocks dealt to the 8 cores so every core holds 20 blocks and
block-slot bi has a uniform edge depth K[bi] across cores (same NEFF on
all cores).  Blocks run in ascending-K order.

Edge layout per block: slot (j*128 + d) holds the j-th in-edge of dst d,
so dst d's edges live on partition d and er needs no per-edge gather
(per-partition broadcast).  Blocks are gathered in chunks of <= KC j's
(dma_gather on alternating SWDGE queues; descriptor generation runs on
both Q7 cores).  Per chunk: s = el_src + er_dst -> Prelu(0.2) -> Exp
(Scalar engine, bf16 out), denominator accumulates on DVE, msg =
h_src * ee (DVE), then PSUM-accumulating matmuls with an identity
stationary.  At block end: out = elu(psum * (1/den)) with
elu(x) = relu(x) - relu(1 - exp(x)).  The emission is software-pipelined
with a one-chunk skew so the in-order DVE queue never parks on scalar
results.

Pad slots gather a "kill" row (el = -1e9 -> ee = 0); pad dst nodes
gather a "neutral" row (el = 0) so their denominator stays positive.

Layer-2 dense (h2 @ W2 and its el/er via the host-precomputed
W2 @ ALCAT) is fused into the layer-1 edge loop through a PE transpose
of each finished output block, and the FC layer is fused into the
layer-2 edge loop the same way.  Biases are all zero in this problem
and are skipped.
"""

import numpy as np
import ml_dtypes

import concourse.bass as bass
import concourse.bacc as bacc
import concourse.mybir as mybir
import concourse.tile as tile
from concourse.bass_utils import run_bass_kernel_spmd

F32 = mybir.dt.float32
BF16 = mybir.dt.bfloat16
I16 = mybir.dt.int16
AF = mybir.ActivationFunctionType
ALU = mybir.AluOpType

# ---------------------------------------------------------------- config ---
N_NODES = 20000
N_CORES = 8
IN_F, OUT_F, HEADS = 1280, 128, 4
HID = OUT_F * HEADS  # 512
FC_O = 64

BLK = 128                            # dst nodes per block
N_BLOCKS = 20                        # blocks per core
N_PAD = N_BLOCKS * BLK               # 2560 local rows per core
N_GBLK = N_CORES * N_BLOCKS          # 160 blocks total
G_ROWS = N_CORES * N_PAD             # 20480 rows in gathered node array
ROW_ELEMS = 640                      # bf16 elems per row: 512 h + 8 (el f32) + pad
K1 = IN_F // 128                     # 10 contraction tiles layer 1
K2 = HID // 128                      # 4  contraction tiles layer 2
KC = 12                              # max j-depth per gather chunk
NEG = -1.0e9


def _wrap_idx(v):
    """dma_gather index layout: [128, n/16] int16 (16-wrap, replicated)."""
    assert len(v) % 16 == 0
    w = v.reshape(-1, 16).T.astype(np.int16)
    return np.tile(w, (8, 1))


def _preprocess(feature, src, dst):
    src = np.asarray(src).astype(np.int64)
    dst = np.asarray(dst).astype(np.int64)

    deg = np.bincount(dst, minlength=N_NODES).astype(np.int64)
    order = np.argsort(-deg, kind="stable")

    blocks = [order[i * BLK:(i + 1) * BLK] for i in range(N_GBLK)]
    kb = np.array([max(int(deg[b].max()) if len(b) else 0, 1)
                   for b in blocks], np.int64)

    # deal blocks to cores: block-octet i (desc by k) -> slot; slots run
    # ascending K on device, so reverse the octet order
    bo = np.argsort(-kb, kind="stable")
    K = []
    core_blocks = [[] for _ in range(N_CORES)]
    for sl in range(N_BLOCKS):
        grp = bo[(N_BLOCKS - 1 - sl) * N_CORES:(N_BLOCKS - sl) * N_CORES]
        K.append(int(kb[grp].max()))
        for c in range(N_CORES):
            core_blocks[c].append(blocks[grp[c]])

    node_core = np.full(N_NODES, -1, np.int64)
    node_loc = np.full(N_NODES, -1, np.int64)
    for c in range(N_CORES):
        for bi in range(N_BLOCKS):
            b = core_blocks[c][bi]
            for p, n in enumerate(b):
                node_core[n] = c
                node_loc[n] = bi * BLK + p

    # global kill / neutral rows (pad slots in the gathered array)
    pad_rows = []
    for c in range(N_CORES):
        for bi in range(N_BLOCKS):
            used = len(core_blocks[c][bi])
            for p in range(used, BLK):
                pad_rows.append((c, bi * BLK + p))
    assert len(pad_rows) >= 2, "need kill+neutral pad rows"
    kill_c, kill_l = pad_rows[0]
    neut_c, neut_l = pad_rows[1]
    kill_ga = kill_c * N_PAD + kill_l
    neut_ga = neut_c * N_PAD + neut_l

    e_dst_loc = node_loc[dst]
    e_dst_core = node_core[dst]
    e_src_ga = node_core[src] * N_PAD + node_loc[src]

    in_maps_part = []
    for c in range(N_CORES):
        sel = np.nonzero(e_dst_core == c)[0]
        dloc = e_dst_loc[sel]
        sga = e_src_ga[sel]
        o2 = np.argsort(dloc, kind="stable")
        dloc, sga = dloc[o2], sga[o2]
        cnt = np.bincount(dloc, minlength=N_PAD)
        starts = np.zeros(N_PAD, np.int64)
        starts[1:] = np.cumsum(cnt)[:-1]
        jidx = np.arange(len(dloc)) - starts[dloc]

        idxs = []
        for bi in range(N_BLOCKS):
            k = K[bi]
            slots = np.full(k * BLK, kill_ga, np.int64)
            m = (dloc >= bi * BLK) & (dloc < (bi + 1) * BLK)
            d_b = dloc[m] - bi * BLK
            j_b = jidx[m]
            assert j_b.max(initial=0) < k, (bi, k, j_b.max())
            slots[j_b * BLK + d_b] = sga[m]
            used = len(core_blocks[c][bi])
            if used < BLK:
                for p in range(used, BLK):
                    slots[p::BLK] = neut_ga
            idxs.append(_wrap_idx(slots.astype(np.int16)))

        idx_cat = np.concatenate([w.reshape(128, -1) for w in idxs], axis=1)

        mask = np.ones((BLK, N_BLOCKS, 8), np.float32)
        offs = np.zeros((BLK, N_BLOCKS, 8), np.float32)
        for bi in range(N_BLOCKS):
            used = len(core_blocks[c][bi])
            for p in range(used, BLK):
                mask[p, bi, :] = 0.0
                if not (c == neut_c and bi * BLK + p == neut_l):
                    offs[p, bi, 0:4] = NEG

        x_c = np.zeros((N_PAD, IN_F), np.float32)
        for bi in range(N_BLOCKS):
            b = core_blocks[c][bi]
            x_c[bi * BLK:bi * BLK + len(b)] = feature[b]
        xT = np.ascontiguousarray(x_c.T).astype(ml_dtypes.bfloat16)
        in_maps_part.append(dict(xT=xT, idx=idx_cat, melr=mask, oelr=offs))

    unperm = np.zeros(N_NODES, np.int64)
    for c in range(N_CORES):
        for bi in range(N_BLOCKS):
            b = core_blocks[c][bi]
            for p, n in enumerate(b):
                unperm[n] = c * N_PAD + bi * BLK + p
    return in_maps_part, tuple(K), unperm


def _rep(v, parts=128):
    v = np.asarray(v, np.float32).ravel()
    return np.tile(v[None, :], (parts, 1)).astype(np.float32)


def _make_consts(W1, al1, ar1, b1, W2, al2, ar2, b2, Wfc, bfc):
    bf = ml_dtypes.bfloat16
    # ALCAT[hd*128+f, s*4+hd] = al_s[hd, f]; el/er of layer 2 computed on
    # the PE as h2 @ (W2 @ ALCAT) using the already-transposed h2 tiles
    alcat = np.zeros((HID, 8), np.float32)
    for hd in range(HEADS):
        alcat[hd * OUT_F:(hd + 1) * OUT_F, hd] = np.asarray(al2)[hd]
        alcat[hd * OUT_F:(hd + 1) * OUT_F, 4 + hd] = np.asarray(ar2)[hd]
    w2al = np.asarray(W2, np.float32) @ alcat                 # (512, 8)
    return {
        "w1": np.ascontiguousarray(W1).astype(bf),
        "w2": np.ascontiguousarray(W2).astype(bf),
        "w2al": np.ascontiguousarray(w2al).astype(bf),
        "wfc": np.ascontiguousarray(Wfc).astype(bf),
        "alr1": np.concatenate([_rep(al1), _rep(ar1)], 1),
        "ident": np.eye(128, dtype=np.float32).astype(bf),
    }


def _chunks_of(k):
    n = -(-k // KC)
    base = k // n
    rem = k - base * n
    out = []
    j0 = 0
    for i in range(n):
        kc = base + (1 if i < rem else 0)
        out.append((j0, kc))
        j0 += kc
    return out


# ---------------------------------------------------------------- device ---

def build_nc(K):
    IDX_COLS = sum(K) * 8
    nc = bacc.Bacc(
        "TRN2", target_bir_lowering=False, debug=False,
        num_devices=N_CORES, num_swdge_queues=2,
    )

    xT = nc.dram_tensor("xT", [IN_F, N_PAD], BF16, kind="ExternalInput")
    w1 = nc.dram_tensor("w1", [IN_F, HID], BF16, kind="ExternalInput")
    w2 = nc.dram_tensor("w2", [HID, HID], BF16, kind="ExternalInput")
    w2al = nc.dram_tensor("w2al", [HID, 8], BF16, kind="ExternalInput")
    wfc = nc.dram_tensor("wfc", [HID, FC_O], BF16, kind="ExternalInput")
    alr1 = nc.dram_tensor("alr1", [128, 2 * HID], F32, kind="ExternalInput")
    ident_d = nc.dram_tensor("ident", [128, 128], BF16, kind="ExternalInput")
    idx_d = nc.dram_tensor("idx", [128, IDX_COLS], I16, kind="ExternalInput")
    melr_d = nc.dram_tensor("melr", [128, N_BLOCKS, 8], F32,
                            kind="ExternalInput")
    oelr_d = nc.dram_tensor("oelr", [128, N_BLOCKS, 8], F32,
                            kind="ExternalInput")
    out_d = nc.dram_tensor("out", [N_PAD, FC_O], F32, kind="ExternalOutput")

    with tile.TileContext(nc) as tc:
        with tc.tile_pool(name="dram", bufs=1, space="DRAM") as dram:
            na1l = dram.tile([N_PAD, ROW_ELEMS], BF16, name="na1l")
            na1g = dram.tile([G_ROWS, ROW_ELEMS], BF16, name="na1g",
                             addr_space="Shared")
            na2l = dram.tile([N_PAD, ROW_ELEMS], BF16, name="na2l")
            na2g = dram.tile([G_ROWS, ROW_ELEMS], BF16, name="na2g",
                             addr_space="Shared")

            with tc.tile_pool(name="const", bufs=1) as cpool:
                ident_t = cpool.tile([128, 128], BF16, name="ident_t")
                nc.sync.dma_start(ident_t[:, :], ident_d[:, :])
                alr1_t = cpool.tile([128, 2 * HID], F32, name="alr1_t")
                nc.sync.dma_start(alr1_t[:, :], alr1[:, :])
                w2_t = cpool.tile([128, K2, HID], BF16, name="w2_t")
                nc.sync.dma_start(
                    w2_t[:, :, :],
                    w2[:, :].rearrange("(k p) n -> p k n", p=128),
                )
                w2al_t = cpool.tile([128, K2, 8], BF16, name="w2al_t")
                nc.sync.dma_start(
                    w2al_t[:, :, :],
                    w2al[:, :].rearrange("(k p) n -> p k n", p=128),
                )
                wfc_t = cpool.tile([128, K2, FC_O], BF16, name="wfc_t")
                nc.sync.dma_start(
                    wfc_t[:, :, :],
                    wfc[:, :].rearrange("(k p) n -> p k n", p=128),
                )
                idx_t = cpool.tile([128, IDX_COLS], I16, name="idx_t")
                nc.sync.dma_start(idx_t[:, :], idx_d[:, :])
                melr_t = cpool.tile([128, N_BLOCKS, 8], F32, name="melr_t")
                nc.sync.dma_start(melr_t[:, :, :], melr_d[:, :, :])
                oelr_t = cpool.tile([128, N_BLOCKS, 8], F32, name="oelr_t")
                nc.sync.dma_start(oelr_t[:, :, :], oelr_d[:, :, :])
                er1_t = cpool.tile([128, N_BLOCKS, 4], F32, name="er1_t")
                er2_t = cpool.tile([128, N_BLOCKS, 4], F32, name="er2_t")

                _dense1(nc, tc, xT, w1, alr1_t, na1l, er1_t, melr_t,
                        oelr_t)
                _ag(nc, na1l, na1g)
                _edge(nc, tc, K, na_g=na1g, er_t=er1_t, idx_t=idx_t,
                      ident_t=ident_t,
                      d2=(w2_t, w2al_t, na2l, er2_t, melr_t, oelr_t),
                      fc=None)
                _ag(nc, na2l, na2g)
                _edge(nc, tc, K, na_g=na2g, er_t=er2_t, idx_t=idx_t,
                      ident_t=ident_t, d2=None, fc=(wfc_t, out_d))
    nc.compile()
    return nc


def _ag(nc, nal, nag):
    nc.gpsimd.collective_compute(
        "AllGather",
        ALU.bypass,
        replica_groups=[list(range(N_CORES))],
        ins=[nal[:, :].opt()],
        outs=[nag[:, :].opt()],
    )


def _dense1(nc, tc, xT, w1, alr_t, nal, er_t, melr_t, oelr_t):
    """h1 = x @ W1; el/er; node rows [h|el] -> nal; er -> resident tile."""
    with (
        tc.tile_pool(name="d1_lhs", bufs=1) as lhs_pool,
        tc.tile_pool(name="d1_w", bufs=1) as w_pool,
        tc.tile_pool(name="d1_sb", bufs=3) as sb,
        tc.tile_pool(name="d1_ps", bufs=2, space="PSUM") as ps,
    ):
        lhsT = []
        for kt in range(K1):
            t = lhs_pool.tile([128, N_PAD], BF16, name=f"lhsT{kt}")
            nc.sync.dma_start(t[:, :], xT[kt * 128:(kt + 1) * 128, :])
            lhsT.append(t)
        w_t = w_pool.tile([128, K1, HID], BF16, name="w_t")
        nc.sync.dma_start(
            w_t[:, :, :],
            w1[:, :].rearrange("(k p) n -> p k n", p=128),
        )

        for nt in range(N_BLOCKS):
            psum_h = ps.tile([128, HID], F32, name="psum_h")
            for kt in range(K1):
                nc.tensor.matmul(
                    psum_h[:, :],
                    lhsT[kt][:, nt * 128:(nt + 1) * 128],
                    w_t[:, kt, :],
                    start=(kt == 0),
                    stop=(kt == K1 - 1),
                )
            hbf = sb.tile([128, HID], BF16, name="hbf")
            nc.scalar.activation(hbf[:, :], psum_h[:, :], AF.Copy)
            elr = sb.tile([128, 8], F32, name="elr")
            scr = sb.tile([128, 2 * HID], F32, name="ttr_scr")
            nc.vector.tensor_tensor(
                scr[:, :].rearrange("p (s h f) -> p s h f", s=2, h=HEADS),
                psum_h[:, :].rearrange("p (h f) -> p h f", h=HEADS)
                .unsqueeze(1).broadcast_to((128, 2, HEADS, 128)),
                alr_t[:, :].rearrange("p (s h f) -> p s h f", s=2, h=HEADS),
                ALU.mult,
            )
            nc.vector.tensor_reduce(
                elr[:, :],
                scr[:, :].rearrange("p (g f) -> p g f", f=128),
                mybir.AxisListType.X,
                ALU.add,
            )
            _elmask_store(nc, sb, elr, melr_t, oelr_t, nt, er_t, nal, hbf)


def _elmask_store(nc, sb, elr, melr_t, oelr_t, nt, er_t, nal, hbf):
    """elr -> mask+offs -> er tile + [h|el] row writes for node tile nt."""
    elm = sb.tile([128, 8], F32, name="elm")
    nc.vector.tensor_tensor(
        elm[:, :], elr[:, :], melr_t[:, nt, :], ALU.mult
    )
    elo = sb.tile([128, 8], F32, name="elo")
    nc.vector.tensor_tensor(
        elo[:, :], elm[:, :], oelr_t[:, nt, :], ALU.add
    )
    nc.vector.tensor_copy(er_t[:, nt, :], elo[:, 4:8])
    r = nt * 128
    nc.sync.dma_start(nal[r:r + 128, 0:HID], hbf[:, :])
    nal_f32 = nal[:, :].bitcast(F32)
    nc.sync.dma_start(nal_f32[r:r + 128, 256:260], elo[:, 0:4])


def _edge(nc, tc, K, na_g, er_t, idx_t, ident_t, d2, fc):
    """Edge stage; d2 fuses the layer-2 dense, fc fuses the final FC."""
    # flat chunk list across blocks
    chunks = []
    icol = 0
    for bi in range(N_BLOCKS):
        parts = _chunks_of(K[bi])
        for ci, (j0, kc) in enumerate(parts):
            chunks.append(dict(
                bi=bi, j0=j0, kc=kc, icol=icol,
                first=(ci == 0), last=(ci == len(parts) - 1),
            ))
        icol += 8 * K[bi]
    NCH = len(chunks)

    with (
        tc.tile_pool(name="e_ga", bufs=8) as pga,
        tc.tile_pool(name="e_sm", bufs=3) as psm,
        tc.tile_pool(name="e_bk", bufs=2) as pbk,
        tc.tile_pool(name="e_ps", bufs=2, space="PSUM") as pps,
        tc.tile_pool(name="e_pst", bufs=2, space="PSUM") as pst,
        tc.tile_pool(name="e_ps2", bufs=2, space="PSUM") as ps2,
    ):
        state = {}   # per live chunk t -> dict of tiles
        bstate = {}  # per block bi -> dict (den tile, psum_o, ...)

        def emit_gather(t):
            ch = chunks[t]
            kc = ch["kc"]
            gA = pga.tile([128, kc, ROW_ELEMS], BF16, name="gA")
            c0 = ch["icol"] + 8 * ch["j0"]
            nc.gpsimd.dma_gather(
                gA[:, :, :], na_g[:, :], idx_t[:, c0:c0 + 8 * kc],
                kc * BLK, kc * BLK, ROW_ELEMS, single_packet=False,
                queue_num=t % 2,
            )
            state[t] = dict(gA=gA)

        def emit_attn(t):
            ch = chunks[t]
            kc, bi = ch["kc"], ch["bi"]
            gA = state[t]["gA"]
            el_src = gA[:, :, 512:520].bitcast(F32)   # (128, kc, 4)
            # lr[p,h,j] = prelu(el[p,j,h] + er[p,h], 0.2), fused on Scalar
            lr_t = psm.tile([128, 4, kc], F32, name="lr_t")
            for h in range(HEADS):
                nc.scalar.activation(
                    lr_t[:, h, :], el_src[:, :, h], AF.Prelu,
                    bias=er_t[:, bi, h:h + 1], alpha=0.2,
                )
            ee_t = psm.tile([128, 4, kc], BF16, name="ee_t")
            nc.scalar.activation(ee_t[:, :, :], lr_t[:, :, :], AF.Exp)
            state[t]["ee"] = ee_t

        def emit_msg(t):
            ch = chunks[t]
            kc, bi = ch["kc"], ch["bi"]
            gA, ee_t = state[t]["gA"], state[t]["ee"]
            if ch["first"]:
                den = pbk.tile([128, 4], F32, name="den")
                psum_o = pps.tile([128, HID], F32, name="psum_o")
                bstate[bi] = dict(den=den, psum_o=psum_o)
            den = bstate[bi]["den"]
            psum_o = bstate[bi]["psum_o"]
            # in place: gA h-columns *= ee (broadcast over f)
            nc.vector.tensor_tensor(
                gA[:, :, 0:HID].rearrange("p j (h f) -> p j h f",
                                          h=HEADS),
                gA[:, :, 0:HID].rearrange("p j (h f) -> p j h f",
                                          h=HEADS),
                ee_t[:, :, :].rearrange("p h j -> p j h").unsqueeze(3)
                .broadcast_to((128, kc, HEADS, OUT_F)),
                ALU.mult,
            )
            if ch["first"]:
                nc.vector.tensor_reduce(
                    den[:, :], ee_t[:, :, :], mybir.AxisListType.X,
                    ALU.add,
                )
            else:
                dc = psm.tile([128, 4], F32, name="dc")
                nc.vector.tensor_reduce(
                    dc[:, :], ee_t[:, :, :], mybir.AxisListType.X,
                    ALU.add,
                )
                nc.vector.tensor_tensor(
                    den[:, :], den[:, :], dc[:, :], ALU.add
                )
            for j in range(kc):
                nc.tensor.matmul(
                    psum_o[:, :], ident_t[:, :], gA[:, j, 0:HID],
                    start=(ch["first"] and j == 0),
                    stop=(ch["last"] and j == kc - 1),
                )
            del state[t]

        def emit_head(bi):
            # out = elu(psum/den): DVE divide + scalar elu pieces
            st = bstate[bi]
            den, psum_o = st["den"], st["psum_o"]
            rec = pbk.tile([128, 4], F32, name="rec")
            nc.vector.reciprocal(rec[:, :], den[:, :])
            o1 = pbk.tile([128, HID], F32, name="o1")
            nc.vector.tensor_tensor(
                o1[:, :].rearrange("p (h f) -> p h f", h=HEADS),
                psum_o[:, :].rearrange("p (h f) -> p h f", h=HEADS),
                rec[:, :].unsqueeze(2).broadcast_to((128, HEADS, OUT_F)),
                ALU.mult,
            )
            ex = pbk.tile([128, HID], F32, name="ex")
            nc.scalar.activation(ex[:, :], o1[:, :], AF.Exp)
            r1 = pbk.tile([128, HID], F32, name="r1")
            nc.scalar.activation(r1[:, :], ex[:, :], AF.Relu, bias=1.0,
                                 scale=-1.0)
            rl = pbk.tile([128, HID], F32, name="rl")
            nc.scalar.activation(rl[:, :], o1[:, :], AF.Relu)
            st["r1"], st["rl"] = r1, rl

        def emit_tail(bi):
            st = bstate.pop(bi)
            hn = pbk.tile([128, HID], BF16, name="hn")
            nc.vector.tensor_tensor(hn[:, :], st["rl"], st["r1"],
                                    ALU.subtract)
            if d2 is not None:
                w2_t, w2al_t, na2l, er2_t, melr_t, oelr_t = d2
                lt = pbk.tile([128, K2, 128], BF16, name="lt2")
                for kt in range(K2):
                    ptr = pst.tile([128, 128], BF16, name="ptr")
                    nc.tensor.transpose(
                        ptr[:, :], hn[:, kt * 128:(kt + 1) * 128],
                        ident_t[:, :],
                    )
                    nc.scalar.activation(lt[:, kt, :], ptr[:, :], AF.Copy)
                psum_h2 = ps2.tile([128, HID], F32, name="psum_h2")
                for kt in range(K2):
                    nc.tensor.matmul(
                        psum_h2[:, :], lt[:, kt, :], w2_t[:, kt, :],
                        start=(kt == 0), stop=(kt == K2 - 1),
                    )
                psum_el = pst.tile([128, 8], F32, name="psum_el")
                for kt in range(K2):
                    nc.tensor.matmul(
                        psum_el[:, :], lt[:, kt, :], w2al_t[:, kt, :],
                        start=(kt == 0), stop=(kt == K2 - 1),
                    )
                hbf2 = pbk.tile([128, HID], BF16, name="hbf2")
                nc.scalar.activation(hbf2[:, :], psum_h2[:, :], AF.Copy)
                elr2 = pbk.tile([128, 8], F32, name="elr2")
                nc.scalar.activation(elr2[:, :], psum_el[:, :], AF.Copy)
                _elmask_store(nc, pbk, elr2, melr_t, oelr_t, bi, er2_t,
                              na2l, hbf2)
            if fc is not None:
                wfc_t, out_d = fc
                lt = pbk.tile([128, K2, 128], BF16, name="ltf")
                for kt in range(K2):
                    ptr = pst.tile([128, 128], BF16, name="ptr")
                    nc.tensor.transpose(
                        ptr[:, :], hn[:, kt * 128:(kt + 1) * 128],
                        ident_t[:, :],
                    )
                    nc.scalar.activation(lt[:, kt, :], ptr[:, :], AF.Copy)
                psum_f = ps2.tile([128, FC_O], F32, name="psum_f")
                for kt in range(K2):
                    nc.tensor.matmul(
                        psum_f[:, :], lt[:, kt, :], wfc_t[:, kt, :],
                        start=(kt == 0), stop=(kt == K2 - 1),
                    )
                fex = pbk.tile([128, FC_O], F32, name="fex")
                nc.scalar.activation(fex[:, :], psum_f[:, :], AF.Exp)
                fr1 = pbk.tile([128, FC_O], F32, name="fr1")
                nc.scalar.activation(fr1[:, :], fex[:, :], AF.Relu,
                                     bias=1.0, scale=-1.0)
                frl = pbk.tile([128, FC_O], F32, name="frl")
                nc.scalar.activation(frl[:, :], psum_f[:, :], AF.Relu)
                outf = pbk.tile([128, FC_O], F32, name="outf")
                nc.vector.tensor_tensor(
                    outf[:, :], frl[:, :], fr1[:, :], ALU.subtract
                )
                nc.sync.dma_start(
                    out_d[bi * 128:(bi + 1) * 128, :], outf[:, :]
                )

        # software-pipelined emission: gather(t), attn(t-1), msg(t-2);
        # block head after its last msg, block tail one chunk later
        pend_head = []
        pend_tail = []
        for t in range(NCH + 2):
            if t < NCH:
                emit_gather(t)
            if 0 <= t - 1 < NCH:
                emit_attn(t - 1)
            if 0 <= t - 2 < NCH:
                while pend_tail:
                    emit_tail(pend_tail.pop(0))
                emit_msg(t - 2)
                ch = chunks[t - 2]
                if ch["last"]:
                    emit_head(ch["bi"])
                    pend_tail.append(ch["bi"])
        while pend_tail:
            emit_tail(pend_tail.pop(0))


# ------------------------------------------------------------------ host ---

_CACHE = {}
LAST_RESULT = None  # BassKernelResults of the most recent run (for test.py)


def kernel(feature, W1, al1, ar1, b1, W2, al2, ar2, b2, Wfc, bfc, src, dst):
    assert not np.any(np.asarray(b1)) and not np.any(np.asarray(b2)) \
        and not np.any(np.asarray(bfc)), "kernel assumes zero biases"
    feature = np.asarray(feature, np.float32)
    in_parts, K, unperm = _preprocess(feature, src, dst)
    consts = _make_consts(W1, al1, ar1, b1, W2, al2, ar2, b2, Wfc, bfc)

    if K not in _CACHE:
        _CACHE[K] = build_nc(K)
    nc = _CACHE[K]

    in_maps = []
    for c in range(N_CORES):
        m = dict(consts)
        m.update(in_parts[c])
        in_maps.append(m)

    res = run_bass_kernel_spmd(nc, in_maps, core_ids=list(range(N_CORES)))
    global LAST_RESULT
    LAST_RESULT = res
    allout = np.concatenate(
        [np.asarray(res.results[c]["out"]) for c in range(N_CORES)], axis=0
    )
    out = allout[unperm][None, :, :].astype(np.float32)
    return out


# revision 39
# speedup vs baseline: 1.0790x; 1.0790x over previous
"""GAT 2-layer + FC Trainium2 kernel, 8-core SPMD — degree-sorted edition.

Sharding: nodes sorted by in-degree, grouped into 160 blocks of 128 dst
nodes; blocks dealt to the 8 cores so every core holds 20 blocks and
block-slot bi has a uniform edge depth K[bi] across cores (same NEFF on
all cores).  Blocks run in ascending-K order.

Edge layout per block: slot (j*128 + d) holds the j-th in-edge of dst d,
so dst d's edges live on partition d and er needs no per-edge gather
(per-partition broadcast).  Blocks are gathered in chunks of <= KC j's
(dma_gather on alternating SWDGE queues; descriptor generation runs on
both Q7 cores).  Per chunk: s = el_src + er_dst -> Prelu(0.2) -> Exp
(Scalar engine, bf16 out), denominator accumulates on DVE, msg =
h_src * ee (DVE), then PSUM-accumulating matmuls with an identity
stationary.  At block end: out = elu(psum * (1/den)) with
elu(x) = relu(x) - relu(1 - exp(x)).  The emission is software-pipelined
with a one-chunk skew so the in-order DVE queue never parks on scalar
results.

Pad slots gather a "kill" row (el = -1e9 -> ee = 0); pad dst nodes
gather a "neutral" row (el = 0) so their denominator stays positive.

Layer-2 dense (h2 @ W2 and its el/er via the host-precomputed
W2 @ ALCAT) is fused into the layer-1 edge loop through a PE transpose
of each finished output block, and the FC layer is fused into the
layer-2 edge loop the same way.  Biases are all zero in this problem
and are skipped.
"""

import numpy as np
import ml_dtypes

import concourse.bass as bass
import concourse.bacc as bacc
import concourse.mybir as mybir
import concourse.tile as tile
from concourse.bass_utils import run_bass_kernel_spmd

F32 = mybir.dt.float32
BF16 = mybir.dt.bfloat16
I16 = mybir.dt.int16
AF = mybir.ActivationFunctionType
ALU = mybir.AluOpType

# ---------------------------------------------------------------- config ---
N_NODES = 20000
N_CORES = 8
IN_F, OUT_F, HEADS = 1280, 128, 4
HID = OUT_F * HEADS  # 512
FC_O = 64

BLK = 128                            # dst nodes per block
N_BLOCKS = 20                        # blocks per core
N_PAD = N_BLOCKS * BLK               # 2560 local rows per core
N_GBLK = N_CORES * N_BLOCKS          # 160 blocks total
G_ROWS = N_CORES * N_PAD             # 20480 rows in gathered node array
ROW_ELEMS = 640                      # bf16 elems per row: 512 h + 8 (el f32) + pad
K1 = IN_F // 128                     # 10 contraction tiles layer 1
K2 = HID // 128                      # 4  contraction tiles layer 2
KC = 12                              # max j-depth per gather chunk
NEG = -1.0e9


def _wrap_idx(v):
    """dma_gather index layout: [128, n/16] int16 (16-wrap, replicated)."""
    assert len(v) % 16 == 0
    w = v.reshape(-1, 16).T.astype(np.int16)
    return np.tile(w, (8, 1))


def _preprocess(feature, src, dst):
    src = np.asarray(src).astype(np.int64)
    dst = np.asarray(dst).astype(np.int64)

    deg = np.bincount(dst, minlength=N_NODES).astype(np.int64)
    order = np.argsort(-deg, kind="stable")

    blocks = [order[i * BLK:(i + 1) * BLK] for i in range(N_GBLK)]
    kb = np.array([max(int(deg[b].max()) if len(b) else 0, 1)
                   for b in blocks], np.int64)

    # deal blocks to cores: block-octet i (desc by k) -> slot; slots run
    # ascending K on device, so reverse the octet order
    bo = np.argsort(-kb, kind="stable")
    K = []
    core_blocks = [[] for _ in range(N_CORES)]
    for sl in range(N_BLOCKS):
        grp = bo[(N_BLOCKS - 1 - sl) * N_CORES:(N_BLOCKS - sl) * N_CORES]
        K.append(int(kb[grp].max()))
        for c in range(N_CORES):
            core_blocks[c].append(blocks[grp[c]])

    node_core = np.full(N_NODES, -1, np.int64)
    node_loc = np.full(N_NODES, -1, np.int64)
    for c in range(N_CORES):
        for bi in range(N_BLOCKS):
            b = core_blocks[c][bi]
            for p, n in enumerate(b):
                node_core[n] = c
                node_loc[n] = bi * BLK + p

    # global kill / neutral rows (pad slots in the gathered array)
    pad_rows = []
    for c in range(N_CORES):
        for bi in range(N_BLOCKS):
            used = len(core_blocks[c][bi])
            for p in range(used, BLK):
                pad_rows.append((c, bi * BLK + p))
    assert len(pad_rows) >= 2, "need kill+neutral pad rows"
    kill_c, kill_l = pad_rows[0]
    neut_c, neut_l = pad_rows[1]
    kill_ga = kill_c * N_PAD + kill_l
    neut_ga = neut_c * N_PAD + neut_l

    e_dst_loc = node_loc[dst]
    e_dst_core = node_core[dst]
    e_src_ga = node_core[src] * N_PAD + node_loc[src]

    in_maps_part = []
    for c in range(N_CORES):
        sel = np.nonzero(e_dst_core == c)[0]
        dloc = e_dst_loc[sel]
        sga = e_src_ga[sel]
        o2 = np.argsort(dloc, kind="stable")
        dloc, sga = dloc[o2], sga[o2]
        cnt = np.bincount(dloc, minlength=N_PAD)
        starts = np.zeros(N_PAD, np.int64)
        starts[1:] = np.cumsum(cnt)[:-1]
        jidx = np.arange(len(dloc)) - starts[dloc]

        idxs = []
        for bi in range(N_BLOCKS):
            k = K[bi]
            slots = np.full(k * BLK, kill_ga, np.int64)
            m = (dloc >= bi * BLK) & (dloc < (bi + 1) * BLK)
            d_b = dloc[m] - bi * BLK
            j_b = jidx[m]
            assert j_b.max(initial=0) < k, (bi, k, j_b.max())
            slots[j_b * BLK + d_b] = sga[m]
            used = len(core_blocks[c][bi])
            if used < BLK:
                for p in range(used, BLK):
                    slots[p::BLK] = neut_ga
            idxs.append(_wrap_idx(slots.astype(np.int16)))

        idx_cat = np.concatenate([w.reshape(128, -1) for w in idxs], axis=1)

        mask = np.ones((BLK, N_BLOCKS, 8), np.float32)
        offs = np.zeros((BLK, N_BLOCKS, 8), np.float32)
        for bi in range(N_BLOCKS):
            used = len(core_blocks[c][bi])
            for p in range(used, BLK):
                mask[p, bi, :] = 0.0
                if not (c == neut_c and bi * BLK + p == neut_l):
                    offs[p, bi, 0:4] = NEG

        x_c = np.zeros((N_PAD, IN_F), np.float32)
        for bi in range(N_BLOCKS):
            b = core_blocks[c][bi]
            x_c[bi * BLK:bi * BLK + len(b)] = feature[b]
        xT = np.ascontiguousarray(x_c.T).astype(ml_dtypes.bfloat16)
        in_maps_part.append(dict(xT=xT, idx=idx_cat, melr=mask, oelr=offs))

    unperm = np.zeros(N_NODES, np.int64)
    for c in range(N_CORES):
        for bi in range(N_BLOCKS):
            b = core_blocks[c][bi]
            for p, n in enumerate(b):
                unperm[n] = c * N_PAD + bi * BLK + p
    return in_maps_part, tuple(K), unperm


def _rep(v, parts=128):
    v = np.asarray(v, np.float32).ravel()
    return np.tile(v[None, :], (parts, 1)).astype(np.float32)


def _make_consts(W1, al1, ar1, b1, W2, al2, ar2, b2, Wfc, bfc):
    bf = ml_dtypes.bfloat16
    # ALCAT[hd*128+f, s*4+hd] = al_s[hd, f]; el/er of layer 2 computed on
    # the PE as h2 @ (W2 @ ALCAT) using the already-transposed h2 tiles
    alcat = np.zeros((HID, 8), np.float32)
    for hd in range(HEADS):
        alcat[hd * OUT_F:(hd + 1) * OUT_F, hd] = np.asarray(al2)[hd]
        alcat[hd * OUT_F:(hd + 1) * OUT_F, 4 + hd] = np.asarray(ar2)[hd]
    w2al = np.asarray(W2, np.float32) @ alcat                 # (512, 8)
    return {
        "w1": np.ascontiguousarray(W1).astype(bf),
        "w2": np.ascontiguousarray(W2).astype(bf),
        "w2al": np.ascontiguousarray(w2al).astype(bf),
        "wfc": np.ascontiguousarray(Wfc).astype(bf),
        "alr1": np.concatenate([_rep(al1), _rep(ar1)], 1),
        "ident": np.eye(128, dtype=np.float32).astype(bf),
    }


def _chunks_of(k):
    n = -(-k // KC)
    base = k // n
    rem = k - base * n
    out = []
    j0 = 0
    for i in range(n):
        kc = base + (1 if i < rem else 0)
        out.append((j0, kc))
        j0 += kc
    return out


# ---------------------------------------------------------------- device ---

def build_nc(K):
    IDX_COLS = sum(K) * 8
    nc = bacc.Bacc(
        "TRN2", target_bir_lowering=False, debug=False,
        num_devices=N_CORES, num_swdge_queues=2,
    )

    xT = nc.dram_tensor("xT", [IN_F, N_PAD], BF16, kind="ExternalInput")
    w1 = nc.dram_tensor("w1", [IN_F, HID], BF16, kind="ExternalInput")
    w2 = nc.dram_tensor("w2", [HID, HID], BF16, kind="ExternalInput")
    w2al = nc.dram_tensor("w2al", [HID, 8], BF16, kind="ExternalInput")
    wfc = nc.dram_tensor("wfc", [HID, FC_O], BF16, kind="ExternalInput")
    alr1 = nc.dram_tensor("alr1", [128, 2 * HID], F32, kind="ExternalInput")
    ident_d = nc.dram_tensor("ident", [128, 128], BF16, kind="ExternalInput")
    idx_d = nc.dram_tensor("idx", [128, IDX_COLS], I16, kind="ExternalInput")
    melr_d = nc.dram_tensor("melr", [128, N_BLOCKS, 8], F32,
                            kind="ExternalInput")
    oelr_d = nc.dram_tensor("oelr", [128, N_BLOCKS, 8], F32,
                            kind="ExternalInput")
    out_d = nc.dram_tensor("out", [N_PAD, FC_O], F32, kind="ExternalOutput")

    with tile.TileContext(nc) as tc:
        with tc.tile_pool(name="dram", bufs=1, space="DRAM") as dram:
            na1l = dram.tile([N_PAD, ROW_ELEMS], BF16, name="na1l")
            na1g = dram.tile([G_ROWS, ROW_ELEMS], BF16, name="na1g",
                             addr_space="Shared")
            na2l = dram.tile([N_PAD, ROW_ELEMS], BF16, name="na2l")
            na2g = dram.tile([G_ROWS, ROW_ELEMS], BF16, name="na2g",
                             addr_space="Shared")

            with tc.tile_pool(name="const", bufs=1) as cpool:
                ident_t = cpool.tile([128, 128], BF16, name="ident_t")
                nc.sync.dma_start(ident_t[:, :], ident_d[:, :])
                alr1_t = cpool.tile([128, 2 * HID], F32, name="alr1_t")
                nc.sync.dma_start(alr1_t[:, :], alr1[:, :])
                w2_t = cpool.tile([128, K2, HID], BF16, name="w2_t")
                nc.sync.dma_start(
                    w2_t[:, :, :],
                    w2[:, :].rearrange("(k p) n -> p k n", p=128),
                )
                w2al_t = cpool.tile([128, K2, 8], BF16, name="w2al_t")
                nc.sync.dma_start(
                    w2al_t[:, :, :],
                    w2al[:, :].rearrange("(k p) n -> p k n", p=128),
                )
                wfc_t = cpool.tile([128, K2, FC_O], BF16, name="wfc_t")
                nc.sync.dma_start(
                    wfc_t[:, :, :],
                    wfc[:, :].rearrange("(k p) n -> p k n", p=128),
                )
                idx_t = cpool.tile([128, IDX_COLS], I16, name="idx_t")
                nc.sync.dma_start(idx_t[:, :], idx_d[:, :])
                melr_t = cpool.tile([128, N_BLOCKS, 8], F32, name="melr_t")
                nc.sync.dma_start(melr_t[:, :, :], melr_d[:, :, :])
                oelr_t = cpool.tile([128, N_BLOCKS, 8], F32, name="oelr_t")
                nc.sync.dma_start(oelr_t[:, :, :], oelr_d[:, :, :])
                er1_t = cpool.tile([128, N_BLOCKS, 4], F32, name="er1_t")
                er2_t = cpool.tile([128, N_BLOCKS, 4], F32, name="er2_t")

                _dense1(nc, tc, xT, w1, alr1_t, na1l, er1_t, melr_t,
                        oelr_t)
                _ag(nc, na1l, na1g)
                _edge(nc, tc, K, na_g=na1g, er_t=er1_t, idx_t=idx_t,
                      ident_t=ident_t,
                      d2=(w2_t, w2al_t, na2l, er2_t, melr_t, oelr_t),
                      fc=None)
                _ag(nc, na2l, na2g)
                _edge(nc, tc, K, na_g=na2g, er_t=er2_t, idx_t=idx_t,
                      ident_t=ident_t, d2=None, fc=(wfc_t, out_d))
    nc.compile()
    return nc


def _ag(nc, nal, nag):
    nc.gpsimd.collective_compute(
        "AllGather",
        ALU.bypass,
        replica_groups=[list(range(N_CORES))],
        ins=[nal[:, :].opt()],
        outs=[nag[:, :].opt()],
    )


def _dense1(nc, tc, xT, w1, alr_t, nal, er_t, melr_t, oelr_t):
    """h1 = x @ W1; el/er; node rows [h|el] -> nal; er -> resident tile."""
    with (
        tc.tile_pool(name="d1_lhs", bufs=1) as lhs_pool,
        tc.tile_pool(name="d1_w", bufs=1) as w_pool,
        tc.tile_pool(name="d1_sb", bufs=3) as sb,
        tc.tile_pool(name="d1_ps", bufs=2, space="PSUM") as ps,
    ):
        lhsT = []
        for kt in range(K1):
            t = lhs_pool.tile([128, N_PAD], BF16, name=f"lhsT{kt}")
            nc.sync.dma_start(t[:, :], xT[kt * 128:(kt + 1) * 128, :])
            lhsT.append(t)
        w_t = w_pool.tile([128, K1, HID], BF16, name="w_t")
        nc.sync.dma_start(
            w_t[:, :, :],
            w1[:, :].rearrange("(k p) n -> p k n", p=128),
        )

        for nt in range(N_BLOCKS):
            psum_h = ps.tile([128, HID], F32, name="psum_h")
            for kt in range(K1):
                nc.tensor.matmul(
                    psum_h[:, :],
                    lhsT[kt][:, nt * 128:(nt + 1) * 128],
                    w_t[:, kt, :],
                    start=(kt == 0),
                    stop=(kt == K1 - 1),
                )
            hbf = sb.tile([128, HID], BF16, name="hbf")
            nc.scalar.activation(hbf[:, :], psum_h[:, :], AF.Copy)
            elr = sb.tile([128, 8], F32, name="elr")
            scr = sb.tile([128, 2 * HID], F32, name="ttr_scr")
            nc.vector.tensor_tensor(
                scr[:, :].rearrange("p (s h f) -> p s h f", s=2, h=HEADS),
                psum_h[:, :].rearrange("p (h f) -> p h f", h=HEADS)
                .unsqueeze(1).broadcast_to((128, 2, HEADS, 128)),
                alr_t[:, :].rearrange("p (s h f) -> p s h f", s=2, h=HEADS),
                ALU.mult,
            )
            nc.vector.tensor_reduce(
                elr[:, :],
                scr[:, :].rearrange("p (g f) -> p g f", f=128),
                mybir.AxisListType.X,
                ALU.add,
            )
            _elmask_store(nc, sb, elr, melr_t, oelr_t, nt, er_t, nal, hbf)


def _elmask_store(nc, sb, elr, melr_t, oelr_t, nt, er_t, nal, hbf):
    """elr -> mask+offs -> er tile + [h|el] row writes for node tile nt."""
    elm = sb.tile([128, 8], F32, name="elm")
    nc.vector.tensor_tensor(
        elm[:, :], elr[:, :], melr_t[:, nt, :], ALU.mult
    )
    elo = sb.tile([128, 8], F32, name="elo")
    nc.vector.tensor_tensor(
        elo[:, :], elm[:, :], oelr_t[:, nt, :], ALU.add
    )
    nc.vector.tensor_copy(er_t[:, nt, :], elo[:, 4:8])
    r = nt * 128
    nc.sync.dma_start(nal[r:r + 128, 0:HID], hbf[:, :])
    nal_f32 = nal[:, :].bitcast(F32)
    nc.sync.dma_start(nal_f32[r:r + 128, 256:260], elo[:, 0:4])


def _edge(nc, tc, K, na_g, er_t, idx_t, ident_t, d2, fc):
    """Edge stage; d2 fuses the layer-2 dense, fc fuses the final FC."""
    # flat chunk list across blocks
    chunks = []
    icol = 0
    for bi in range(N_BLOCKS):
        parts = _chunks_of(K[bi])
        for ci, (j0, kc) in enumerate(parts):
            chunks.append(dict(
                bi=bi, j0=j0, kc=kc, icol=icol,
                first=(ci == 0), last=(ci == len(parts) - 1),
            ))
        icol += 8 * K[bi]
    NCH = len(chunks)

    with (
        tc.tile_pool(name="e_ga", bufs=8) as pga,
        tc.tile_pool(name="e_sm", bufs=3) as psm,
        tc.tile_pool(name="e_bk", bufs=2) as pbk,
        tc.tile_pool(name="e_ps", bufs=2, space="PSUM") as pps,
        tc.tile_pool(name="e_pst", bufs=2, space="PSUM") as pst,
        tc.tile_pool(name="e_ps2", bufs=2, space="PSUM") as ps2,
    ):
        state = {}   # per live chunk t -> dict of tiles
        bstate = {}  # per block bi -> dict (den tile, psum_o, ...)

        def emit_gather(t):
            ch = chunks[t]
            kc = ch["kc"]
            gA = pga.tile([128, kc, ROW_ELEMS], BF16, name="gA")
            c0 = ch["icol"] + 8 * ch["j0"]
            nc.gpsimd.dma_gather(
                gA[:, :, :], na_g[:, :], idx_t[:, c0:c0 + 8 * kc],
                kc * BLK, kc * BLK, ROW_ELEMS, single_packet=False,
                queue_num=t % 2,
            )
            state[t] = dict(gA=gA)

        def emit_attn(t):
            ch = chunks[t]
            kc, bi = ch["kc"], ch["bi"]
            gA = state[t]["gA"]
            el_src = gA[:, :, 512:520].bitcast(F32)   # (128, kc, 4)
            s_t = psm.tile([128, 4, kc], F32, name="s_t")
            nc.vector.tensor_tensor(
                s_t[:, :, :],
                el_src.rearrange("p j f -> p f j"),
                er_t[:, bi, :].unsqueeze(2).broadcast_to((128, 4, kc)),
                ALU.add,
            )
            lr_t = psm.tile([128, 4, kc], F32, name="lr_t")
            nc.scalar.activation(lr_t[:, :, :], s_t[:, :, :], AF.Prelu,
                                 alpha=0.2)
            ee_t = psm.tile([128, 4, kc], BF16, name="ee_t")
            nc.scalar.activation(ee_t[:, :, :], lr_t[:, :, :], AF.Exp)
            state[t]["ee"] = ee_t

        def emit_msg(t):
            ch = chunks[t]
            kc, bi = ch["kc"], ch["bi"]
            gA, ee_t = state[t]["gA"], state[t]["ee"]
            if ch["first"]:
                den = pbk.tile([128, 4], F32, name="den")
                psum_o = pps.tile([128, HID], F32, name="psum_o")
                bstate[bi] = dict(den=den, psum_o=psum_o)
            den = bstate[bi]["den"]
            psum_o = bstate[bi]["psum_o"]
            # in place: gA h-columns *= ee (broadcast over f)
            nc.vector.tensor_tensor(
                gA[:, :, 0:HID].rearrange("p j (h f) -> p j h f",
                                          h=HEADS),
                gA[:, :, 0:HID].rearrange("p j (h f) -> p j h f",
                                          h=HEADS),
                ee_t[:, :, :].rearrange("p h j -> p j h").unsqueeze(3)
                .broadcast_to((128, kc, HEADS, OUT_F)),
                ALU.mult,
            )
            if ch["first"]:
                nc.vector.tensor_reduce(
                    den[:, :], ee_t[:, :, :], mybir.AxisListType.X,
                    ALU.add,
                )
            else:
                dc = psm.tile([128, 4], F32, name="dc")
                nc.vector.tensor_reduce(
                    dc[:, :], ee_t[:, :, :], mybir.AxisListType.X,
                    ALU.add,
                )
                nc.vector.tensor_tensor(
                    den[:, :], den[:, :], dc[:, :], ALU.add
                )
            for j in range(kc):
                nc.tensor.matmul(
                    psum_o[:, :], ident_t[:, :], gA[:, j, 0:HID],
                    start=(ch["first"] and j == 0),
                    stop=(ch["last"] and j == kc - 1),
                )
            del state[t]

        def emit_head(bi):
            # out = elu(psum/den): DVE divide + scalar elu pieces
            st = bstate[bi]
            den, psum_o = st["den"], st["psum_o"]
            rec = pbk.tile([128, 4], F32, name="rec")
            nc.vector.reciprocal(rec[:, :], den[:, :])
            o1 = pbk.tile([128, HID], F32, name="o1")
            nc.vector.tensor_tensor(
                o1[:, :].rearrange("p (h f) -> p h f", h=HEADS),
                psum_o[:, :].rearrange("p (h f) -> p h f", h=HEADS),
                rec[:, :].unsqueeze(2).broadcast_to((128, HEADS, OUT_F)),
                ALU.mult,
            )
            ex = pbk.tile([128, HID], F32, name="ex")
            nc.scalar.activation(ex[:, :], o1[:, :], AF.Exp)
            r1 = pbk.tile([128, HID], F32, name="r1")
            nc.scalar.activation(r1[:, :], ex[:, :], AF.Relu, bias=1.0,
                                 scale=-1.0)
            rl = pbk.tile([128, HID], F32, name="rl")
            nc.scalar.activation(rl[:, :], o1[:, :], AF.Relu)
            st["r1"], st["rl"] = r1, rl

        def emit_tail(bi):
            st = bstate.pop(bi)
            hn = pbk.tile([128, HID], BF16, name="hn")
            nc.vector.tensor_tensor(hn[:, :], st["rl"], st["r1"],
                                    ALU.subtract)
            if d2 is not None:
                w2_t, w2al_t, na2l, er2_t, melr_t, oelr_t = d2
                lt = pbk.tile([128, K2, 128], BF16, name="lt2")
                for kt in range(K2):
                    ptr = pst.tile([128, 128], BF16, name="ptr")
                    nc.tensor.transpose(
                        ptr[:, :], hn[:, kt * 128:(kt + 1) * 128],
                        ident_t[:, :],
                    )
                    nc.scalar.activation(lt[:, kt, :], ptr[:, :], AF.Copy)
                psum_h2 = ps2.tile([128, HID], F32, name="psum_h2")
                for kt in range(K2):
                    nc.tensor.matmul(
                        psum_h2[:, :], lt[:, kt, :], w2_t[:, kt, :],
                        start=(kt == 0), stop=(kt == K2 - 1),
                    )
                psum_el = pst.tile([128, 8], F32, name="psum_el")
                for kt in range(K2):
                    nc.tensor.matmul(
                        psum_el[:, :], lt[:, kt, :], w2al_t[:, kt, :],
                        start=(kt == 0), stop=(kt == K2 - 1),
                    )
                hbf2 = pbk.tile([128, HID], BF16, name="hbf2")
                nc.scalar.activation(hbf2[:, :], psum_h2[:, :], AF.Copy)
                elr2 = pbk.tile([128, 8], F32, name="elr2")
                nc.scalar.activation(elr2[:, :], psum_el[:, :], AF.Copy)
                _elmask_store(nc, pbk, elr2, melr_t, oelr_t, bi, er2_t,
                              na2l, hbf2)
            if fc is not None:
                wfc_t, out_d = fc
                lt = pbk.tile([128, K2, 128], BF16, name="ltf")
                for kt in range(K2):
                    ptr = pst.tile([128, 128], BF16, name="ptr")
                    nc.tensor.transpose(
                        ptr[:, :], hn[:, kt * 128:(kt + 1) * 128],
                        ident_t[:, :],
                    )
                    nc.scalar.activation(lt[:, kt, :], ptr[:, :], AF.Copy)
                psum_f = ps2.tile([128, FC_O], F32, name="psum_f")
                for kt in range(K2):
                    nc.tensor.matmul(
                        psum_f[:, :], lt[:, kt, :], wfc_t[:, kt, :],
                        start=(kt == 0), stop=(kt == K2 - 1),
                    )
                fex = pbk.tile([128, FC_O], F32, name="fex")
                nc.scalar.activation(fex[:, :], psum_f[:, :], AF.Exp)
                fr1 = pbk.tile([128, FC_O], F32, name="fr1")
                nc.scalar.activation(fr1[:, :], fex[:, :], AF.Relu,
                                     bias=1.0, scale=-1.0)
                frl = pbk.tile([128, FC_O], F32, name="frl")
                nc.scalar.activation(frl[:, :], psum_f[:, :], AF.Relu)
                outf = pbk.tile([128, FC_O], F32, name="outf")
                nc.vector.tensor_tensor(
                    outf[:, :], frl[:, :], fr1[:, :], ALU.subtract
                )
                nc.sync.dma_start(
                    out_d[bi * 128:(bi + 1) * 128, :], outf[:, :]
                )

        # software-pipelined emission: gather(t), attn(t-1), msg(t-2);
        # block head after its last msg, block tail one chunk later
        pend_head = []
        pend_tail = []
        for t in range(NCH + 2):
            if t < NCH:
                emit_gather(t)
            if 0 <= t - 1 < NCH:
                emit_attn(t - 1)
            if 0 <= t - 2 < NCH:
                while pend_tail:
                    emit_tail(pend_tail.pop(0))
                emit_msg(t - 2)
                ch = chunks[t - 2]
                if ch["last"]:
                    emit_head(ch["bi"])
                    pend_tail.append(ch["bi"])
        while pend_tail:
            emit_tail(pend_tail.pop(0))


# ------------------------------------------------------------------ host ---

_CACHE = {}
LAST_RESULT = None  # BassKernelResults of the most recent run (for test.py)


def kernel(feature, W1, al1, ar1, b1, W2, al2, ar2, b2, Wfc, bfc, src, dst):
    assert not np.any(np.asarray(b1)) and not np.any(np.asarray(b2)) \
        and not np.any(np.asarray(bfc)), "kernel assumes zero biases"
    feature = np.asarray(feature, np.float32)
    in_parts, K, unperm = _preprocess(feature, src, dst)
    consts = _make_consts(W1, al1, ar1, b1, W2, al2, ar2, b2, Wfc, bfc)

    if K not in _CACHE:
        _CACHE[K] = build_nc(K)
    nc = _CACHE[K]

    in_maps = []
    for c in range(N_CORES):
        m = dict(consts)
        m.update(in_parts[c])
        in_maps.append(m)

    res = run_bass_kernel_spmd(nc, in_maps, core_ids=list(range(N_CORES)))
    global LAST_RESULT
    LAST_RESULT = res
    allout = np.concatenate(
        [np.asarray(res.results[c]["out"]) for c in range(N_CORES)], axis=0
    )
    out = allout[unperm][None, :, :].astype(np.float32)
    return out


# revision 42
# speedup vs baseline: 1.1270x; 1.0445x over previous
"""GAT 2-layer + FC Trainium2 kernel, 8-core SPMD — degree-sorted edition.

Sharding: nodes sorted by in-degree, grouped into 160 blocks of 128 dst
nodes; blocks dealt to the 8 cores so every core holds 20 blocks and
block-slot bi has a uniform edge depth K[bi] across cores (same NEFF on
all cores).  Blocks run in ascending-K order.

Edge layout per block: slot (j*128 + d) holds the j-th in-edge of dst d,
so dst d's edges live on partition d and er needs no per-edge gather
(per-partition broadcast).  Blocks are gathered in chunks of <= KC j's
(dma_gather on alternating SWDGE queues; descriptor generation runs on
both Q7 cores).  Per chunk: s = el_src + er_dst -> Prelu(0.2) -> Exp
(Scalar engine, bf16 out), denominator accumulates on DVE, msg =
h_src * ee (DVE), then PSUM-accumulating matmuls with an identity
stationary.  At block end: out = elu(psum * (1/den)) with
elu(x) = relu(x) - relu(1 - exp(x)).  The emission is software-pipelined
with a one-chunk skew so the in-order DVE queue never parks on scalar
results.

Pad slots gather a "kill" row (el = -1e9 -> ee = 0); pad dst nodes
gather a "neutral" row (el = 0) so their denominator stays positive.

Layer-2 dense (h2 @ W2 and its el/er via the host-precomputed
W2 @ ALCAT) is fused into the layer-1 edge loop through a PE transpose
of each finished output block, and the FC layer is fused into the
layer-2 edge loop the same way.  Biases are all zero in this problem
and are skipped.
"""

import numpy as np
import ml_dtypes

import concourse.bass as bass
import concourse.bacc as bacc
import concourse.mybir as mybir
import concourse.tile as tile
from concourse.bass_utils import run_bass_kernel_spmd

F32 = mybir.dt.float32
BF16 = mybir.dt.bfloat16
I16 = mybir.dt.int16
AF = mybir.ActivationFunctionType
ALU = mybir.AluOpType

# ---------------------------------------------------------------- config ---
N_NODES = 20000
N_CORES = 8
IN_F, OUT_F, HEADS = 1280, 128, 4
HID = OUT_F * HEADS  # 512
FC_O = 64

BLK = 128                            # dst nodes per block
N_BLOCKS = 20                        # blocks per core
N_PAD = N_BLOCKS * BLK               # 2560 local rows per core
N_GBLK = N_CORES * N_BLOCKS          # 160 blocks total
G_ROWS = N_CORES * N_PAD             # 20480 rows in gathered node array
ROW_ELEMS = 640                      # bf16 elems per row: 512 h + 8 (el f32) + pad
K1 = IN_F // 128                     # 10 contraction tiles layer 1
K2 = HID // 128                      # 4  contraction tiles layer 2
KC = 10                              # max j-depth per gather chunk
NEG = -1.0e9


def _wrap_idx(v):
    """dma_gather index layout: [128, n/16] int16 (16-wrap, replicated)."""
    assert len(v) % 16 == 0
    w = v.reshape(-1, 16).T.astype(np.int16)
    return np.tile(w, (8, 1))


def _preprocess(feature, src, dst):
    src = np.asarray(src).astype(np.int64)
    dst = np.asarray(dst).astype(np.int64)

    deg = np.bincount(dst, minlength=N_NODES).astype(np.int64)
    order = np.argsort(-deg, kind="stable")

    blocks = [order[i * BLK:(i + 1) * BLK] for i in range(N_GBLK)]
    kb = np.array([max(int(deg[b].max()) if len(b) else 0, 1)
                   for b in blocks], np.int64)

    # deal blocks to cores: block-octet i (desc by k) -> slot; slots run
    # ascending K on device, so reverse the octet order
    bo = np.argsort(-kb, kind="stable")
    K = []
    core_blocks = [[] for _ in range(N_CORES)]
    for sl in range(N_BLOCKS):
        grp = bo[(N_BLOCKS - 1 - sl) * N_CORES:(N_BLOCKS - sl) * N_CORES]
        K.append(int(kb[grp].max()))
        for c in range(N_CORES):
            core_blocks[c].append(blocks[grp[c]])

    node_core = np.full(N_NODES, -1, np.int64)
    node_loc = np.full(N_NODES, -1, np.int64)
    for c in range(N_CORES):
        for bi in range(N_BLOCKS):
            b = core_blocks[c][bi]
            for p, n in enumerate(b):
                node_core[n] = c
                node_loc[n] = bi * BLK + p

    # global kill / neutral rows (pad slots in the gathered array)
    pad_rows = []
    for c in range(N_CORES):
        for bi in range(N_BLOCKS):
            used = len(core_blocks[c][bi])
            for p in range(used, BLK):
                pad_rows.append((c, bi * BLK + p))
    assert len(pad_rows) >= 2, "need kill+neutral pad rows"
    kill_c, kill_l = pad_rows[0]
    neut_c, neut_l = pad_rows[1]
    kill_ga = kill_c * N_PAD + kill_l
    neut_ga = neut_c * N_PAD + neut_l

    e_dst_loc = node_loc[dst]
    e_dst_core = node_core[dst]
    e_src_ga = node_core[src] * N_PAD + node_loc[src]

    in_maps_part = []
    for c in range(N_CORES):
        sel = np.nonzero(e_dst_core == c)[0]
        dloc = e_dst_loc[sel]
        sga = e_src_ga[sel]
        o2 = np.argsort(dloc, kind="stable")
        dloc, sga = dloc[o2], sga[o2]
        cnt = np.bincount(dloc, minlength=N_PAD)
        starts = np.zeros(N_PAD, np.int64)
        starts[1:] = np.cumsum(cnt)[:-1]
        jidx = np.arange(len(dloc)) - starts[dloc]

        idxs = []
        for bi in range(N_BLOCKS):
            k = K[bi]
            slots = np.full(k * BLK, kill_ga, np.int64)
            m = (dloc >= bi * BLK) & (dloc < (bi + 1) * BLK)
            d_b = dloc[m] - bi * BLK
            j_b = jidx[m]
            assert j_b.max(initial=0) < k, (bi, k, j_b.max())
            slots[j_b * BLK + d_b] = sga[m]
            used = len(core_blocks[c][bi])
            if used < BLK:
                for p in range(used, BLK):
                    slots[p::BLK] = neut_ga
            idxs.append(_wrap_idx(slots.astype(np.int16)))

        idx_cat = np.concatenate([w.reshape(128, -1) for w in idxs], axis=1)

        mask = np.ones((BLK, N_BLOCKS, 8), np.float32)
        offs = np.zeros((BLK, N_BLOCKS, 8), np.float32)
        for bi in range(N_BLOCKS):
            used = len(core_blocks[c][bi])
            for p in range(used, BLK):
                mask[p, bi, :] = 0.0
                if not (c == neut_c and bi * BLK + p == neut_l):
                    offs[p, bi, 0:4] = NEG

        x_c = np.zeros((N_PAD, IN_F), np.float32)
        for bi in range(N_BLOCKS):
            b = core_blocks[c][bi]
            x_c[bi * BLK:bi * BLK + len(b)] = feature[b]
        xT = np.ascontiguousarray(x_c.T).astype(ml_dtypes.bfloat16)
        in_maps_part.append(dict(xT=xT, idx=idx_cat, melr=mask, oelr=offs))

    unperm = np.zeros(N_NODES, np.int64)
    for c in range(N_CORES):
        for bi in range(N_BLOCKS):
            b = core_blocks[c][bi]
            for p, n in enumerate(b):
                unperm[n] = c * N_PAD + bi * BLK + p
    return in_maps_part, tuple(K), unperm


def _rep(v, parts=128):
    v = np.asarray(v, np.float32).ravel()
    return np.tile(v[None, :], (parts, 1)).astype(np.float32)


def _make_consts(W1, al1, ar1, b1, W2, al2, ar2, b2, Wfc, bfc):
    bf = ml_dtypes.bfloat16
    # ALCAT[hd*128+f, s*4+hd] = al_s[hd, f]; el/er of layer 2 computed on
    # the PE as h2 @ (W2 @ ALCAT) using the already-transposed h2 tiles
    alcat = np.zeros((HID, 8), np.float32)
    for hd in range(HEADS):
        alcat[hd * OUT_F:(hd + 1) * OUT_F, hd] = np.asarray(al2)[hd]
        alcat[hd * OUT_F:(hd + 1) * OUT_F, 4 + hd] = np.asarray(ar2)[hd]
    w2al = np.asarray(W2, np.float32) @ alcat                 # (512, 8)
    return {
        "w1": np.ascontiguousarray(W1).astype(bf),
        "w2": np.ascontiguousarray(W2).astype(bf),
        "w2al": np.ascontiguousarray(w2al).astype(bf),
        "wfc": np.ascontiguousarray(Wfc).astype(bf),
        "alr1": np.concatenate([_rep(al1), _rep(ar1)], 1),
        "ident": np.eye(128, dtype=np.float32).astype(bf),
    }


def _chunks_of(k):
    n = -(-k // KC)
    base = k // n
    rem = k - base * n
    out = []
    j0 = 0
    for i in range(n):
        kc = base + (1 if i < rem else 0)
        out.append((j0, kc))
        j0 += kc
    return out


# ---------------------------------------------------------------- device ---

def build_nc(K):
    IDX_COLS = sum(K) * 8
    nc = bacc.Bacc(
        "TRN2", target_bir_lowering=False, debug=False,
        num_devices=N_CORES, num_swdge_queues=2,
    )

    xT = nc.dram_tensor("xT", [IN_F, N_PAD], BF16, kind="ExternalInput")
    w1 = nc.dram_tensor("w1", [IN_F, HID], BF16, kind="ExternalInput")
    w2 = nc.dram_tensor("w2", [HID, HID], BF16, kind="ExternalInput")
    w2al = nc.dram_tensor("w2al", [HID, 8], BF16, kind="ExternalInput")
    wfc = nc.dram_tensor("wfc", [HID, FC_O], BF16, kind="ExternalInput")
    alr1 = nc.dram_tensor("alr1", [128, 2 * HID], F32, kind="ExternalInput")
    ident_d = nc.dram_tensor("ident", [128, 128], BF16, kind="ExternalInput")
    idx_d = nc.dram_tensor("idx", [128, IDX_COLS], I16, kind="ExternalInput")
    melr_d = nc.dram_tensor("melr", [128, N_BLOCKS, 8], F32,
                            kind="ExternalInput")
    oelr_d = nc.dram_tensor("oelr", [128, N_BLOCKS, 8], F32,
                            kind="ExternalInput")
    out_d = nc.dram_tensor("out", [N_PAD, FC_O], F32, kind="ExternalOutput")

    with tile.TileContext(nc) as tc:
        with tc.tile_pool(name="dram", bufs=1, space="DRAM") as dram:
            na1l = dram.tile([N_PAD, ROW_ELEMS], BF16, name="na1l")
            na1g = dram.tile([G_ROWS, ROW_ELEMS], BF16, name="na1g",
                             addr_space="Shared")
            na2l = dram.tile([N_PAD, ROW_ELEMS], BF16, name="na2l")
            na2g = dram.tile([G_ROWS, ROW_ELEMS], BF16, name="na2g",
                             addr_space="Shared")

            with tc.tile_pool(name="const", bufs=1) as cpool:
                ident_t = cpool.tile([128, 128], BF16, name="ident_t")
                nc.sync.dma_start(ident_t[:, :], ident_d[:, :])
                alr1_t = cpool.tile([128, 2 * HID], F32, name="alr1_t")
                nc.sync.dma_start(alr1_t[:, :], alr1[:, :])
                w2_t = cpool.tile([128, K2, HID], BF16, name="w2_t")
                nc.sync.dma_start(
                    w2_t[:, :, :],
                    w2[:, :].rearrange("(k p) n -> p k n", p=128),
                )
                w2al_t = cpool.tile([128, K2, 8], BF16, name="w2al_t")
                nc.sync.dma_start(
                    w2al_t[:, :, :],
                    w2al[:, :].rearrange("(k p) n -> p k n", p=128),
                )
                wfc_t = cpool.tile([128, K2, FC_O], BF16, name="wfc_t")
                nc.sync.dma_start(
                    wfc_t[:, :, :],
                    wfc[:, :].rearrange("(k p) n -> p k n", p=128),
                )
                idx_t = cpool.tile([128, IDX_COLS], I16, name="idx_t")
                nc.sync.dma_start(idx_t[:, :], idx_d[:, :])
                melr_t = cpool.tile([128, N_BLOCKS, 8], F32, name="melr_t")
                nc.sync.dma_start(melr_t[:, :, :], melr_d[:, :, :])
                oelr_t = cpool.tile([128, N_BLOCKS, 8], F32, name="oelr_t")
                nc.sync.dma_start(oelr_t[:, :, :], oelr_d[:, :, :])
                er1_t = cpool.tile([128, N_BLOCKS, 4], F32, name="er1_t")
                er2_t = cpool.tile([128, N_BLOCKS, 4], F32, name="er2_t")

                _dense1(nc, tc, xT, w1, alr1_t, na1l, er1_t, melr_t,
                        oelr_t)
                _ag(nc, na1l, na1g)
                _edge(nc, tc, K, na_g=na1g, er_t=er1_t, idx_t=idx_t,
                      ident_t=ident_t,
                      d2=(w2_t, w2al_t, na2l, er2_t, melr_t, oelr_t),
                      fc=None)
                _ag(nc, na2l, na2g)
                _edge(nc, tc, K, na_g=na2g, er_t=er2_t, idx_t=idx_t,
                      ident_t=ident_t, d2=None, fc=(wfc_t, out_d))
    nc.compile()
    return nc


def _ag(nc, nal, nag):
    nc.gpsimd.collective_compute(
        "AllGather",
        ALU.bypass,
        replica_groups=[list(range(N_CORES))],
        ins=[nal[:, :].opt()],
        outs=[nag[:, :].opt()],
    )


def _dense1(nc, tc, xT, w1, alr_t, nal, er_t, melr_t, oelr_t):
    """h1 = x @ W1; el/er; node rows [h|el] -> nal; er -> resident tile."""
    with (
        tc.tile_pool(name="d1_lhs", bufs=1) as lhs_pool,
        tc.tile_pool(name="d1_w", bufs=1) as w_pool,
        tc.tile_pool(name="d1_sb", bufs=3) as sb,
        tc.tile_pool(name="d1_ps", bufs=3, space="PSUM") as ps,
    ):
        lhsT = []
        for kt in range(K1):
            t = lhs_pool.tile([128, N_PAD], BF16, name=f"lhsT{kt}")
            nc.sync.dma_start(t[:, :], xT[kt * 128:(kt + 1) * 128, :])
            lhsT.append(t)
        w_t = w_pool.tile([128, K1, HID], BF16, name="w_t")
        nc.sync.dma_start(
            w_t[:, :, :],
            w1[:, :].rearrange("(k p) n -> p k n", p=128),
        )

        for nt in range(N_BLOCKS):
            psum_h = ps.tile([128, HID], F32, name="psum_h")
            for kt in range(K1):
                nc.tensor.matmul(
                    psum_h[:, :],
                    lhsT[kt][:, nt * 128:(nt + 1) * 128],
                    w_t[:, kt, :],
                    start=(kt == 0),
                    stop=(kt == K1 - 1),
                )
            hbf = sb.tile([128, HID], BF16, name="hbf")
            nc.scalar.activation(hbf[:, :], psum_h[:, :], AF.Copy)
            elr = sb.tile([128, 8], F32, name="elr")
            scr = sb.tile([128, 2 * HID], F32, name="ttr_scr")
            nc.vector.tensor_tensor(
                scr[:, :].rearrange("p (s h f) -> p s h f", s=2, h=HEADS),
                psum_h[:, :].rearrange("p (h f) -> p h f", h=HEADS)
                .unsqueeze(1).broadcast_to((128, 2, HEADS, 128)),
                alr_t[:, :].rearrange("p (s h f) -> p s h f", s=2, h=HEADS),
                ALU.mult,
            )
            nc.vector.tensor_reduce(
                elr[:, :],
                scr[:, :].rearrange("p (g f) -> p g f", f=128),
                mybir.AxisListType.X,
                ALU.add,
            )
            _elmask_store(nc, sb, elr, melr_t, oelr_t, nt, er_t, nal, hbf)


def _elmask_store(nc, sb, elr, melr_t, oelr_t, nt, er_t, nal, hbf):
    """elr -> mask+offs -> er tile + [h|el] row writes for node tile nt."""
    elm = sb.tile([128, 8], F32, name="elm")
    nc.vector.tensor_tensor(
        elm[:, :], elr[:, :], melr_t[:, nt, :], ALU.mult
    )
    elo = sb.tile([128, 8], F32, name="elo")
    nc.vector.tensor_tensor(
        elo[:, :], elm[:, :], oelr_t[:, nt, :], ALU.add
    )
    nc.vector.tensor_copy(er_t[:, nt, :], elo[:, 4:8])
    r = nt * 128
    nc.sync.dma_start(nal[r:r + 128, 0:HID], hbf[:, :])
    nal_f32 = nal[:, :].bitcast(F32)
    nc.sync.dma_start(nal_f32[r:r + 128, 256:260], elo[:, 0:4])


def _edge(nc, tc, K, na_g, er_t, idx_t, ident_t, d2, fc):
    """Edge stage; d2 fuses the layer-2 dense, fc fuses the final FC."""
    # flat chunk list across blocks
    chunks = []
    icol = 0
    for bi in range(N_BLOCKS):
        parts = _chunks_of(K[bi])
        for ci, (j0, kc) in enumerate(parts):
            chunks.append(dict(
                bi=bi, j0=j0, kc=kc, icol=icol,
                first=(ci == 0), last=(ci == len(parts) - 1),
            ))
        icol += 8 * K[bi]
    NCH = len(chunks)

    with (
        tc.tile_pool(name="e_ga", bufs=10) as pga,
        tc.tile_pool(name="e_sm", bufs=3) as psm,
        tc.tile_pool(name="e_bk", bufs=2) as pbk,
        tc.tile_pool(name="e_ps", bufs=2, space="PSUM") as pps,
        tc.tile_pool(name="e_pst", bufs=2, space="PSUM") as pst,
        tc.tile_pool(name="e_ps2", bufs=2, space="PSUM") as ps2,
    ):
        state = {}   # per live chunk t -> dict of tiles
        bstate = {}  # per block bi -> dict (den tile, psum_o, ...)

        def emit_gather(t):
            ch = chunks[t]
            kc = ch["kc"]
            gA = pga.tile([128, kc, ROW_ELEMS], BF16, name="gA")
            c0 = ch["icol"] + 8 * ch["j0"]
            nc.gpsimd.dma_gather(
                gA[:, :, :], na_g[:, :], idx_t[:, c0:c0 + 8 * kc],
                kc * BLK, kc * BLK, ROW_ELEMS, single_packet=False,
                queue_num=t % 2,
            )
            state[t] = dict(gA=gA)

        def emit_attn(t):
            ch = chunks[t]
            kc, bi = ch["kc"], ch["bi"]
            gA = state[t]["gA"]
            el_src = gA[:, :, 512:520].bitcast(F32)   # (128, kc, 4)
            s_t = psm.tile([128, 4, kc], F32, name="s_t")
            nc.vector.tensor_tensor(
                s_t[:, :, :],
                el_src.rearrange("p j f -> p f j"),
                er_t[:, bi, :].unsqueeze(2).broadcast_to((128, 4, kc)),
                ALU.add,
            )
            lr_t = psm.tile([128, 4, kc], F32, name="lr_t")
            nc.scalar.activation(lr_t[:, :, :], s_t[:, :, :], AF.Prelu,
                                 alpha=0.2)
            ee_t = psm.tile([128, 4, kc], BF16, name="ee_t")
            nc.scalar.activation(ee_t[:, :, :], lr_t[:, :, :], AF.Exp)
            state[t]["ee"] = ee_t

        def emit_msg(t):
            ch = chunks[t]
            kc, bi = ch["kc"], ch["bi"]
            gA, ee_t = state[t]["gA"], state[t]["ee"]
            if ch["first"]:
                den = pbk.tile([128, 4], F32, name="den")
                psum_o = pps.tile([128, HID], F32, name="psum_o")
                bstate[bi] = dict(den=den, psum_o=psum_o)
            den = bstate[bi]["den"]
            psum_o = bstate[bi]["psum_o"]
            # in place: gA h-columns *= ee (broadcast over f)
            nc.vector.tensor_tensor(
                gA[:, :, 0:HID].rearrange("p j (h f) -> p j h f",
                                          h=HEADS),
                gA[:, :, 0:HID].rearrange("p j (h f) -> p j h f",
                                          h=HEADS),
                ee_t[:, :, :].rearrange("p h j -> p j h").unsqueeze(3)
                .broadcast_to((128, kc, HEADS, OUT_F)),
                ALU.mult,
            )
            if ch["first"]:
                nc.vector.tensor_reduce(
                    den[:, :], ee_t[:, :, :], mybir.AxisListType.X,
                    ALU.add,
                )
            else:
                dc = psm.tile([128, 4], F32, name="dc")
                nc.vector.tensor_reduce(
                    dc[:, :], ee_t[:, :, :], mybir.AxisListType.X,
                    ALU.add,
                )
                nc.vector.tensor_tensor(
                    den[:, :], den[:, :], dc[:, :], ALU.add
                )
            for j in range(kc):
                nc.tensor.matmul(
                    psum_o[:, :], ident_t[:, :], gA[:, j, 0:HID],
                    start=(ch["first"] and j == 0),
                    stop=(ch["last"] and j == kc - 1),
                )
            del state[t]

        def emit_head(bi):
            # out = elu(psum/den): DVE divide + scalar elu pieces
            st = bstate[bi]
            den, psum_o = st["den"], st["psum_o"]
            rec = pbk.tile([128, 4], F32, name="rec")
            nc.vector.reciprocal(rec[:, :], den[:, :])
            o1 = pbk.tile([128, HID], F32, name="o1")
            nc.vector.tensor_tensor(
                o1[:, :].rearrange("p (h f) -> p h f", h=HEADS),
                psum_o[:, :].rearrange("p (h f) -> p h f", h=HEADS),
                rec[:, :].unsqueeze(2).broadcast_to((128, HEADS, OUT_F)),
                ALU.mult,
            )
            ex = pbk.tile([128, HID], F32, name="ex")
            nc.scalar.activation(ex[:, :], o1[:, :], AF.Exp)
            r1 = pbk.tile([128, HID], F32, name="r1")
            nc.scalar.activation(r1[:, :], ex[:, :], AF.Relu, bias=1.0,
                                 scale=-1.0)
            rl = pbk.tile([128, HID], F32, name="rl")
            nc.scalar.activation(rl[:, :], o1[:, :], AF.Relu)
            st["r1"], st["rl"] = r1, rl

        def emit_tail(bi):
            st = bstate.pop(bi)
            hn = pbk.tile([128, HID], BF16, name="hn")
            nc.vector.tensor_tensor(hn[:, :], st["rl"], st["r1"],
                                    ALU.subtract)
            if d2 is not None:
                w2_t, w2al_t, na2l, er2_t, melr_t, oelr_t = d2
                lt = pbk.tile([128, K2, 128], BF16, name="lt2")
                for kt in range(K2):
                    ptr = pst.tile([128, 128], BF16, name="ptr")
                    nc.tensor.transpose(
                        ptr[:, :], hn[:, kt * 128:(kt + 1) * 128],
                        ident_t[:, :],
                    )
                    nc.scalar.activation(lt[:, kt, :], ptr[:, :], AF.Copy)
                psum_h2 = ps2.tile([128, HID], F32, name="psum_h2")
                for kt in range(K2):
                    nc.tensor.matmul(
                        psum_h2[:, :], lt[:, kt, :], w2_t[:, kt, :],
                        start=(kt == 0), stop=(kt == K2 - 1),
                    )
                psum_el = pst.tile([128, 8], F32, name="psum_el")
                for kt in range(K2):
                    nc.tensor.matmul(
                        psum_el[:, :], lt[:, kt, :], w2al_t[:, kt, :],
                        start=(kt == 0), stop=(kt == K2 - 1),
                    )
                hbf2 = pbk.tile([128, HID], BF16, name="hbf2")
                nc.scalar.activation(hbf2[:, :], psum_h2[:, :], AF.Copy)
                elr2 = pbk.tile([128, 8], F32, name="elr2")
                nc.scalar.activation(elr2[:, :], psum_el[:, :], AF.Copy)
                _elmask_store(nc, pbk, elr2, melr_t, oelr_t, bi, er2_t,
                              na2l, hbf2)
            if fc is not None:
                wfc_t, out_d = fc
                lt = pbk.tile([128, K2, 128], BF16, name="ltf")
                for kt in range(K2):
                    ptr = pst.tile([128, 128], BF16, name="ptr")
                    nc.tensor.transpose(
                        ptr[:, :], hn[:, kt * 128:(kt + 1) * 128],
                        ident_t[:, :],
                    )
                    nc.scalar.activation(lt[:, kt, :], ptr[:, :], AF.Copy)
                psum_f = ps2.tile([128, FC_O], F32, name="psum_f")
                for kt in range(K2):
                    nc.tensor.matmul(
                        psum_f[:, :], lt[:, kt, :], wfc_t[:, kt, :],
                        start=(kt == 0), stop=(kt == K2 - 1),
                    )
                fex = pbk.tile([128, FC_O], F32, name="fex")
                nc.scalar.activation(fex[:, :], psum_f[:, :], AF.Exp)
                fr1 = pbk.tile([128, FC_O], F32, name="fr1")
                nc.scalar.activation(fr1[:, :], fex[:, :], AF.Relu,
                                     bias=1.0, scale=-1.0)
                frl = pbk.tile([128, FC_O], F32, name="frl")
                nc.scalar.activation(frl[:, :], psum_f[:, :], AF.Relu)
                outf = pbk.tile([128, FC_O], F32, name="outf")
                nc.vector.tensor_tensor(
                    outf[:, :], frl[:, :], fr1[:, :], ALU.subtract
                )
                nc.sync.dma_start(
                    out_d[bi * 128:(bi + 1) * 128, :], outf[:, :]
                )

        # software-pipelined emission: gather(t), attn(t-1), msg(t-2);
        # block head after its last msg, block tail one chunk later
        pend_head = []
        pend_tail = []
        for t in range(NCH + 2):
            if t < NCH:
                emit_gather(t)
            if 0 <= t - 1 < NCH:
                emit_attn(t - 1)
            if 0 <= t - 2 < NCH:
                while pend_tail:
                    emit_tail(pend_tail.pop(0))
                emit_msg(t - 2)
                ch = chunks[t - 2]
                if ch["last"]:
                    emit_head(ch["bi"])
                    pend_tail.append(ch["bi"])
        while pend_tail:
            emit_tail(pend_tail.pop(0))


# ------------------------------------------------------------------ host ---

_CACHE = {}
LAST_RESULT = None  # BassKernelResults of the most recent run (for test.py)


def kernel(feature, W1, al1, ar1, b1, W2, al2, ar2, b2, Wfc, bfc, src, dst):
    assert not np.any(np.asarray(b1)) and not np.any(np.asarray(b2)) \
        and not np.any(np.asarray(bfc)), "kernel assumes zero biases"
    feature = np.asarray(feature, np.float32)
    in_parts, K, unperm = _preprocess(feature, src, dst)
    consts = _make_consts(W1, al1, ar1, b1, W2, al2, ar2, b2, Wfc, bfc)

    if K not in _CACHE:
        _CACHE[K] = build_nc(K)
    nc = _CACHE[K]

    in_maps = []
    for c in range(N_CORES):
        m = dict(consts)
        m.update(in_parts[c])
        in_maps.append(m)

    res = run_bass_kernel_spmd(nc, in_maps, core_ids=list(range(N_CORES)))
    global LAST_RESULT
    LAST_RESULT = res
    allout = np.concatenate(
        [np.asarray(res.results[c]["out"]) for c in range(N_CORES)], axis=0
    )
    out = allout[unperm][None, :, :].astype(np.float32)
    return out
